# revision 1
# baseline (speedup 1.0000x reference)
"""CNLinkPredictor Trainium2 kernel (fp8 DoubleRow common-neighbor pipeline).

Edge-sharded across 8 NeuronCores (1024 target edges each); x, adj, and the
MLP weights are replicated. Per core:

  A) h = x + MLP(x), finishing in NATURAL (node-partition) layout so the
     result lands directly in the fp8 block-major layout the DoubleRow
     matmul wants for its stationary operand:
       - L1 stays transposed: y1T = relu(W1^T xT) (fp8 weights + moving).
       - L2 flips orientation per 128-node tile: psum[node, c] accumulates
         y1T-chunks as stationary with W2 moving; the bias lands via a K=1
         ones-row x b2-row matmul; relu on ACT; DVE adds the x residual and
         writes fp8 straight into h8[p, T*256:(T+1)*256].
  B) per 128-edge block: one indirect full-row gather per endpoint from a
     column-shuffled adjacency (host prep), uint16 bitwise-AND in 16
     pieces (exact for 0/1 fp8 patterns, runs at the 2-byte DVE rate,
     sliced so it tracks the gathers without head-of-line-blocking the
     stage-A adds on the in-order DVE stream), two uint16 packed
     transposes (half the xbar tile count of bf16), then 64 DoubleRow fp8
     matmuls accumulating xcnT[c, e] directly - the adjacency column
     shuffle makes the transpose pairing k=2p+j line up with h8's block
     slots.
  C) edge MLPs in transposed layout (bf16), in groups of 2 edge blocks so
     the final group's serial mm->act chain is short: xcnT comes straight
     out of B's PSUM (no transpose); beta is folded into xcn_w2/xcn_b2 on
     the host (beta > 0 commutes with relu) and z = u2 + xij never
     materializes - the lin_w1 matmul distributes over the sum in PSUM.

Scheduling notes (the Tile scheduler builds static in-order per-engine
streams from a priority heap, so latency classes must not share a queue):
  - all loads are host-pretiled slabs / packed const tensors on the SP
    queue (HWDGE dispatch costs ~630 ns of serial time per DMA, so fewer
    + bigger transfers matter); the Act queue carries no DMAs at all.
  - stage A's L2 relu runs on DVE (tensor_scalar max) - on Act its
    per-instruction overhead paced the whole pipeline; the rel pool is
    16 deep so the residual adds may lag without stalling PE/Act.
  - xi/xj gathers interleave at odd groups so their descriptor
    generation never delays the adjacency gathers that feed the
    critical cn pipeline.

Hardware pitfalls this kernel works around (carried from the previous
session, all re-validated):
  - walrus accepts at most ONE sync-wait per instruction
    (_apply_tile_patch + _split_multi_waits).
  - Concurrent 4-byte DMA traffic corrupts in-flight 2-byte xbar
    DMA-transposes: every steady-state transfer is <= 2 bytes/element;
    f32/i32 loads happen up front, the single f32 store happens last.
  - xbar transposes need contiguous per-partition destinations.
  - DoubleRow needs a block-major stationary operand (pair step % 16 == 0);
    the byte-interleaved transpose output is only legal as the MOVING
    operand (verified empirically - the ISA check rejects it as weights).
  - PSUM zero regions are 2048 B: accumulation groups sharing a psum tile
    run strictly block-major so a start=True never clobbers a neighbor.
  - GPSIMD cannot access PSUM (BIR verifier; the cost model does not flag
    it) - PSUM->SBUF copies must stay on DVE.
"""

import numpy as np
import ml_dtypes

N = 8192
C = 256
E = 8192
NCORES = 8
EL = E // NCORES          # edges per core
P = 128
NB = EL // P              # edge blocks per core (8)
NCHUNK = N // 256         # 256-node DoubleRow chunks (32)
NT = N // P               # stage-A node tiles (64)
AGRP = 512                # stage-A node group (4 tiles)
NG = N // AGRP            # stage-A groups (16)
CGRP = 2                  # stage-C blocks per group (256 edges)

_CACHE = {}
TRACE = False
LAST_RESULT = None


def _apply_tile_patch():
    """Split the Tile tail-drain's multi-sem wait onto individual SP nops."""
    from concourse.tile import TileContext
    from concourse.vector_clock import ScopedClock

    if getattr(TileContext, "_drain_patched", False):
        return

    def _patched(self, tick_clock, wait_clock):
        nc = self.nc
        collector = nc.sync.nop()
        wait_clock.add_sem_waits(
            collector.ins, ScopedClock({None: tick_clock.global_clock})
        )
        si = collector.ins.sync_info
        waits = list(si.on_wait) if si is not None and si.on_wait else []
        if si is not None and len(waits) > 1:
            name_to_handle = {h.name: h for h in self.sems.allocated().values()}
            si.on_wait = [waits[0]]
            for w in waits[1:]:
                op = {
                    "sem-ge-imm": "sem-ge",
                    "sem-eq-imm": "sem-eq",
                    "sem-le-imm": "sem-le",
                }.get(str(w.wait_mode), "sem-ge")
                nc.sync.nop().wait_op(name_to_handle[w.ant_name], w.wait_value, op)
        nc.sync.drain()
        nc.all_engine_barrier()
        assert self.sems is not None
        popped = nc._tile_sem_poison_stack.pop()
        assert popped is self._sem_poison
        nc.clear_and_free_semaphores(list(self.sems.allocated().values()))
        nc.all_engine_barrier()

    TileContext._drain_and_barrier = _patched
    TileContext._drain_patched = True


def _split_multi_waits(nc):
    """Hoist extra sync-waits onto same-engine NoOps (sequential waits ==
    ANDed waits); this walrus build allows one wait per instruction."""
    import concourse.mybir as mybir

    cnt = 0
    for fn in nc.m.functions:
        for bb in fn.blocks:
            out = []
            for inst in bb.instructions:
                si = getattr(inst, "sync_info", None)
                waits = list(si.on_wait) if si is not None and si.on_wait else []
                if len(waits) > 1:
                    for w in waits[:-1]:
                        nop = mybir.InstNoOp(name=f"ws-{cnt}", ins=[], outs=[])
                        cnt += 1
                        nop.engine = inst.engine
                        nop.sync_info = mybir.SyncInfo(on_wait=[w], on_update=[])
                        out.append(nop)
                    si.on_wait = [waits[-1]]
                out.append(inst)
            bb.instructions = out
    return nc


def _build(split_waits=True):
    import concourse.bass as bass
    import concourse.mybir as mybir
    from concourse.tile import TileContext

    _apply_tile_patch()

    f32 = mybir.dt.float32
    bf16 = mybir.dt.bfloat16
    fp8 = mybir.dt.float8e4
    u16 = mybir.dt.uint16
    i32 = mybir.dt.int32
    Relu = mybir.ActivationFunctionType.Relu
    Ident = mybir.ActivationFunctionType.Identity
    MUL = mybir.AluOpType.mult
    ADD = mybir.AluOpType.add
    AND = mybir.AluOpType.bitwise_and
    DR = mybir.MatmulPerfMode.DoubleRow

    nc = bass.Bass(num_swdge_queues=4, dynamic_dma_scratch_size=32768)

    # host-pretiled: xa8[p, k*N + n] = x[n, k*128 + p] (fp8)
    xa8_d = nc.dram_tensor("xa8", [P, 2 * N], fp8, kind="ExternalInput")
    # host-pretiled: xr8[p, T*C + c] = x[T*128 + p, c] (fp8)
    xr8_d = nc.dram_tensor("xr8t", [P, 2 * N], fp8, kind="ExternalInput")
    x_d = nc.dram_tensor("x", [N, C], bf16, kind="ExternalInput")
    adjs_d = nc.dram_tensor("adjs", [N, N], fp8, kind="ExternalInput")
    idx_d = nc.dram_tensor("idx", [2, EL], i32, kind="ExternalInput")
    # host-packed fp8 stage-A weights: [p, (which 2, k 2, cout 256)] + ones/b2
    wa8_d = nc.dram_tensor("wa8", [P, 4 * C], fp8, kind="ExternalInput")
    onesb2_d = nc.dram_tensor("onesb2", [1, P + C], fp8, kind="ExternalInput")
    # host-packed bf16 stage-C weights: [p, (which 4, k 2, cout 256)]
    wc_d = nc.dram_tensor("wc", [P, 8 * C], bf16, kind="ExternalInput")
    lin_w2_d = nc.dram_tensor("lin_w2", [C, 1], bf16, kind="ExternalInput")
    bnames = ["xlin_b1", "xcn_b1", "xcn_b2", "xij_b", "lin_b1"]
    # host-packed f32: [p, (bias pairs 10, beta 1, lin_b2 1)]
    fpk_d = nc.dram_tensor("fpk", [P, 2 * len(bnames) + 2], f32,
                           kind="ExternalInput")
    out_d = nc.dram_tensor("out", [1, EL], f32, kind="ExternalOutput")

    _swq = [0]

    def _rr(inst):
        q = _swq[0] % 4
        _swq[0] += 1
        if q:
            inst.ins.queue = f"qPoolDynamic{q}"
        return inst

    with TileContext(nc) as tc:
        with (
            tc.tile_pool(name="const", bufs=1) as pK,
            tc.tile_pool(name="h8p", bufs=1) as pH,
            tc.tile_pool(name="adj", bufs=3) as pAdj,
            tc.tile_pool(name="cn", bufs=2) as pCn,
            tc.tile_pool(name="cnT", bufs=NB - 4) as pT,
            tc.tile_pool(name="xcnT", bufs=1) as pXT,
            tc.tile_pool(name="prod", bufs=2) as pPr,
            tc.tile_pool(name="xij", bufs=1) as pXi,
            tc.tile_pool(name="edge", bufs=1) as pC,
        ):
            # ---- constants (f32/i32 first: they must finish before the
            # first 2-byte xbar transpose is in flight) ----
            idx_sb = pK.tile([P, 2 * NB], i32, tag="idx_sb", name="idx_sb")
            nc.sync.dma_start(
                out=idx_sb[:].rearrange("p (t b) -> p t b", t=2),
                in_=idx_d[:, :].rearrange("t (b p) -> p t b", p=P),
            )
            ii = [idx_sb[:, b:b + 1] for b in range(NB)]
            jj = [idx_sb[:, NB + b:NB + b + 1] for b in range(NB)]

            fpk = pK.tile([P, 2 * len(bnames) + 2], f32, tag="fpk",
                          name="fpk")
            nc.sync.dma_start(out=fpk[:], in_=fpk_d[:, :])
            b_sb = {}
            for q, n in enumerate(bnames):
                b_sb[n] = fpk[:, 2 * q:2 * q + 2]
            beta_sb = fpk[:, 10:11]
            lb2_sb = fpk[:, 11:12]

            wa8 = pK.tile([P, 4 * C], fp8, tag="wa8", name="wa8")
            nc.sync.dma_start(out=wa8[:], in_=wa8_d[:, :])
            w1_sb = wa8[:, 0:2 * C]
            w2_sb = wa8[:, 2 * C:4 * C]
            onesb2 = pK.tile([1, P + C], fp8, tag="onesb2", name="onesb2")
            nc.sync.dma_start(out=onesb2[:], in_=onesb2_d[:, :])
            ones_sb = onesb2[:, 0:P]
            b2row_sb = onesb2[:, P:P + C]

            wc_t = pK.tile([P, 8 * C], bf16, tag="wc", name="wc")
            nc.sync.dma_start(out=wc_t[:], in_=wc_d[:, :])
            wC_sb = {}
            for q, n in enumerate(("xcn_w1", "xcn_w2", "xij_w", "lin_w1")):
                wC_sb[n] = [wc_t[:, q * 2 * C:q * 2 * C + C],
                            wc_t[:, q * 2 * C + C:(q + 1) * 2 * C]]
            lw2_t = pK.tile([P, 2], bf16, tag="lin_w2", name="lin_w2t")
            nc.sync.dma_start(
                out=lw2_t[:].rearrange("p (k o) -> p k o", k=2),
                in_=lin_w2_d[:, :].rearrange("(k p) o -> p k o", p=P),
            )
            lw2_sb = [lw2_t[:, 0:1], lw2_t[:, 1:2]]

            # stage-A input slabs, loaded in 4 chunks so the first
            # adjacency gathers interleave on the serial DMA resource
            xa8 = pK.tile([P, 2 * N], fp8, tag="xa8", name="xa8")
            xr8t = pK.tile([P, 2 * N], fp8, tag="xr8t", name="xr8t")
            NCK = 4
            for ck in range(NCK):
                W2N = 2 * N // NCK
                # xa8 is [p, (k 2, n N)]: load k-halves of each node range
                for k in range(2):
                    sl = slice(k * N + ck * (N // NCK),
                               k * N + (ck + 1) * (N // NCK))
                    nc.sync.dma_start(out=xa8[:, sl], in_=xa8_d[:, sl])
                sl = slice(ck * W2N, (ck + 1) * W2N)
                nc.sync.dma_start(out=xr8t[:, sl], in_=xr8_d[:, sl])

            out_row = pK.tile([1, EL], f32, tag="out_row", name="out_row")

            # h8[p, T*256 + c] = h[node 128*T + p, channel c] in fp8.
            # DoubleRow stationary slice (chunk, ch): [p][j: stride 256]
            # [c2: 128 contiguous] at offset chunk*512 + ch*128.
            h8 = pH.tile([P, 2 * N], fp8, tag="h8", name="h8")
            h8_v = h8[:].rearrange(
                "p (ck j ch c2) -> p ck ch j c2", ck=NCHUNK, j=2, ch=2)

            # ---- stage B state ----
            cnT = [None] * NB
            xcnT_sb = [
                pXT.tile([P, EL], bf16, tag=f"xcnT{ch}", name=f"xcnT{ch}")
                for ch in range(2)
            ]

            def b_gather(b, which):
                t = pAdj.tile([P, N], fp8, tag=f"a{which}", name=f"a{which}{b}")
                off = (ii if which == "i" else jj)[b]
                _rr(nc.gpsimd.indirect_dma_start(
                    out=t[:], out_offset=None, in_=adjs_d[:, :],
                    in_offset=bass.IndirectOffsetOnAxis(ap=off[:, :1], axis=0),
                ))
                return t

            NP_AND = 16  # AND pieces per block (256 u16 cols each)
            HP = NP_AND // 2
            cn8_map = {}

            def b_and_piece(b, q, ai, aj):
                # per-half cn8 tiles: [128, 2048] u16, transposed as soon as
                # the half's 4 AND pieces are done
                half = q // HP
                if q % HP == 0:
                    cn8_map[(b, half)] = pCn.tile(
                        [P, N // 4], u16, tag="cn8", name=f"cn8_{b}_{half}")
                    if half == 0:
                        cnT[b] = pT.tile([P, N // 2], u16, tag="cnT",
                                         name=f"cnT{b}")
                W = (N // 4) // HP
                sl = slice((q % HP) * W, (q % HP + 1) * W)
                base = half * (N // 4)
                nc.vector.tensor_tensor(
                    out=cn8_map[(b, half)][:, sl],
                    in0=ai[:].bitcast(u16)[:, base + sl.start:base + sl.stop],
                    in1=aj[:].bitcast(u16)[:, base + sl.start:base + sl.stop],
                    op=AND,
                )
                if q % HP == HP - 1:
                    cn8 = cn8_map.pop((b, half))
                    nc.sync.dma_start_transpose(
                        out=cnT[b][:, half * (N // 4):(half + 1) * (N // 4)]
                        .rearrange("p (cl e) -> p cl e", e=P),
                        in_=cn8[:],
                    )

            def b_matmuls(b, psT):
                rhs_v = cnT[b][:].bitcast(fp8).rearrange(
                    "p (ck e j) -> p ck j e", ck=NCHUNK, j=2)
                for ch in range(2):
                    o = psT[ch][:, b * P:(b + 1) * P]
                    for chunk in range(NCHUNK):
                        nc.tensor.matmul(
                            o, h8_v[:, chunk, ch, :, :], rhs_v[:, chunk, :, :],
                            start=(chunk == 0), stop=(chunk == NCHUNK - 1),
                            perf_mode=DR,
                        )

            def b_copyout(b, psT):
                for ch in range(2):
                    nc.vector.tensor_copy(
                        xcnT_sb[ch][:, b * P:(b + 1) * P],
                        psT[ch][:, b * P:(b + 1) * P],
                    )

            # ---- stage C (transposed-layout edge MLPs, bf16) ----
            prodT_map = {}
            xi_map = {}

            def c_gathers(b):
                xi = pXi.tile([P, C], bf16, tag=f"xi{b}", name=f"xi{b}")
                _rr(nc.gpsimd.indirect_dma_start(
                    out=xi[:], out_offset=None, in_=x_d[:, :],
                    in_offset=bass.IndirectOffsetOnAxis(
                        ap=ii[b][:, :1], axis=0),
                ))
                xj = pXi.tile([P, C], bf16, tag=f"xj{b}", name=f"xj{b}")
                _rr(nc.gpsimd.indirect_dma_start(
                    out=xj[:], out_offset=None, in_=x_d[:, :],
                    in_offset=bass.IndirectOffsetOnAxis(
                        ap=jj[b][:, :1], axis=0),
                ))
                xi_map[b] = (xi, xj)

            def stage_c_prod(grp):
                W = CGRP * P
                prodT = pPr.tile([P, 2 * W], bf16, tag="prodT", name=f"prodT{grp}")
                prodT_v = prodT[:].rearrange(
                    "p (blk hh e) -> p blk hh e", blk=CGRP, e=P)
                prodT_map[grp] = prodT
                for t2, b in enumerate(range(grp * CGRP, (grp + 1) * CGRP)):
                    xi, xj = xi_map[b]
                    pt = pPr.tile([P, C], bf16, tag="prod", name=f"prod{b}")
                    nc.vector.tensor_tensor(
                        out=pt[:], in0=xi[:], in1=xj[:], op=MUL
                    )
                    nc.sync.dma_start_transpose(
                        out=prodT_v[:, t2, :, :], in_=pt[:],
                    )

            def stage_c(grp, psC, psO):
                W = CGRP * P  # 512 edges

                def mlp_layer(pair, wname, bname, outtag, packed=False):
                    outs = []
                    for h in range(2):
                        ps = psC.tile([P, W], f32, tag="psc",
                                      name=f"psc_{grp}_{outtag}{h}")
                        if packed:
                            rhs_v = pair[:].rearrange(
                                "p (blk hh e) -> p blk hh e", blk=CGRP, e=P)
                            r0, r1 = rhs_v[:, :, 0, :], rhs_v[:, :, 1, :]
                        else:
                            r0, r1 = pair
                        nc.tensor.matmul(
                            ps[:], wC_sb[wname][0][:, h * P:(h + 1) * P],
                            r0, start=True, stop=False,
                        )
                        nc.tensor.matmul(
                            ps[:], wC_sb[wname][1][:, h * P:(h + 1) * P],
                            r1, start=False, stop=True,
                        )
                        t = pC.tile([P, W], bf16, tag=f"{outtag}{h}",
                                    name=f"{outtag}{h}_{grp}")
                        if grp == NB // CGRP - 1:
                            nc.vector.tensor_scalar(
                                t[:], ps[:], b_sb[bname][:, h:h + 1], 0.0,
                                ADD, mybir.AluOpType.max)
                        else:
                            nc.scalar.activation(
                                t[:], ps[:], Relu,
                                bias=b_sb[bname][:, h:h + 1])
                        outs.append(t)
                    return outs

                xijT = mlp_layer(prodT_map[grp], "xij_w", "xij_b", "xijT",
                                 packed=True)
                xcn_pair = (xcnT_sb[0][:, grp * W:(grp + 1) * W],
                            xcnT_sb[1][:, grp * W:(grp + 1) * W])
                u1T = mlp_layer(xcn_pair, "xcn_w1", "xcn_b1", "u1T")
                # beta is folded into xcn_w2/xcn_b2 on the host (beta > 0
                # commutes with relu), and z = u2 + xij never materializes:
                # the lin_w1 matmul distributes over the sum in PSUM.
                u2T = mlp_layer([u1T[0][:], u1T[1][:]], "xcn_w2", "xcn_b2",
                                "u2T")
                vT = []
                for h in range(2):
                    ps = psC.tile([P, W], f32, tag="psc",
                                  name=f"psc_{grp}_vT{h}")
                    nc.tensor.matmul(
                        ps[:], wC_sb["lin_w1"][0][:, h * P:(h + 1) * P],
                        u2T[0][:], start=True, stop=False,
                    )
                    nc.tensor.matmul(
                        ps[:], wC_sb["lin_w1"][1][:, h * P:(h + 1) * P],
                        u2T[1][:], start=False, stop=False,
                    )
                    nc.tensor.matmul(
                        ps[:], wC_sb["lin_w1"][0][:, h * P:(h + 1) * P],
                        xijT[0][:], start=False, stop=False,
                    )
                    nc.tensor.matmul(
                        ps[:], wC_sb["lin_w1"][1][:, h * P:(h + 1) * P],
                        xijT[1][:], start=False, stop=True,
                    )
                    t = pC.tile([P, W], bf16, tag=f"vT{h}",
                                name=f"vT{h}_{grp}")
                    if grp == NB // CGRP - 1:
                        nc.vector.tensor_scalar(
                            t[:], ps[:], b_sb["lin_b1"][:, h:h + 1], 0.0,
                            ADD, mybir.AluOpType.max)
                    else:
                        nc.scalar.activation(
                            t[:], ps[:], Relu, bias=b_sb["lin_b1"][:, h:h + 1])
                    vT.append(t)
                pso = psO.tile([1, W], f32, tag="pso", name=f"pso{grp}")
                nc.tensor.matmul(
                    pso[:], lw2_sb[0][:], vT[0][:], start=True, stop=False
                )
                nc.tensor.matmul(
                    pso[:], lw2_sb[1][:], vT[1][:], start=False, stop=True
                )
                nc.scalar.activation(
                    out_row[0:1, grp * W:(grp + 1) * W], pso[:],
                    Ident, bias=lb2_sb[0:1, 0:1],
                )

            # ---- emission ----
            if True:
                gathered = {}
                with tc.tile_pool(name="stA", bufs=3) as pA, \
                     tc.tile_pool(name="relp", bufs=16) as pRel, \
                     tc.tile_pool(name="psA", bufs=4, space="PSUM") as psA, \
                     tc.tile_pool(name="psL2", bufs=4, space="PSUM") as psL2:
                    and_q = []
                    for g in range(NG):
                        if g % 2 == 1:
                            c_gathers(g // 2)
                        if g % 2 == 0:
                            b = g // 2
                            gathered[b] = (b_gather(b, "i"),
                                           b_gather(b, "j"))
                            for q in range(NP_AND):
                                and_q.append((b, q))

                        # AND pieces first: they track the gathers; the
                        # adds behind them can lag (rel pool absorbs it)
                        navail = (g // 2 + 1) * NP_AND
                        emitted = NP_AND * NB - len(and_q)
                        budget = NP_AND
                        while and_q and budget > 0 and emitted < navail:
                            b2, q2 = and_q.pop(0)
                            b_and_piece(b2, q2, *gathered[b2])
                            emitted += 1
                            budget -= 1

                        m0 = g * AGRP
                        y1T = []
                        for ch in range(2):
                            ps = psA.tile([P, AGRP], f32, tag="psA",
                                          name=f"psA_{g}{ch}")
                            nc.tensor.matmul(
                                ps[:], w1_sb[:, ch * P:(ch + 1) * P],
                                xa8[:, m0:m0 + AGRP],
                                start=True, stop=False,
                            )
                            nc.tensor.matmul(
                                ps[:], w1_sb[:, C + ch * P:C + (ch + 1) * P],
                                xa8[:, N + m0:N + m0 + AGRP],
                                start=False, stop=True,
                            )
                            t = pA.tile([P, AGRP], fp8, tag=f"y1T{ch}",
                                        name=f"y1T{ch}_{g}")
                            nc.scalar.activation(
                                t[:], ps[:], Relu,
                                bias=b_sb["xlin_b1"][:, ch:ch + 1],
                            )
                            y1T.append(t)
                        for t2 in range(4):
                            T = 4 * g + t2
                            ps2 = psL2.tile([P, C], f32, tag="psL2",
                                            name=f"psL2_{T}")
                            nc.tensor.matmul(
                                ps2[:], y1T[0][:, t2 * P:(t2 + 1) * P],
                                w2_sb[:, 0:C], start=True, stop=False,
                            )
                            nc.tensor.matmul(
                                ps2[:], y1T[1][:, t2 * P:(t2 + 1) * P],
                                w2_sb[:, C:2 * C], start=False, stop=False,
                            )
                            nc.tensor.matmul(
                                ps2[:], ones_sb[0:1, :], b2row_sb[0:1, :],
                                start=False, stop=True,
                            )
                            rel = pRel.tile([P, C], bf16, tag="rel",
                                            name=f"rel_{T}")
                            nc.vector.tensor_scalar(
                                rel[:], ps2[:], 0.0, None,
                                mybir.AluOpType.max)
                            nc.vector.tensor_tensor(
                                out=h8[:, T * C:(T + 1) * C],
                                in0=xr8t[:, T * C:(T + 1) * C],
                                in1=rel[:], op=ADD,
                            )
                    while and_q:
                        b2, q2 = and_q.pop(0)
                        b_and_piece(b2, q2, *gathered[b2])

                with tc.tile_pool(name="psB", bufs=1, space="PSUM") as psBp, \
                     tc.tile_pool(name="psC", bufs=3, space="PSUM") as psC, \
                     tc.tile_pool(name="psO", bufs=1, space="PSUM") as psO:
                    psT = [
                        psBp.tile([P, EL], f32, tag=f"psT{ch}",
                                  name=f"psT{ch}")
                        for ch in range(2)
                    ]
                    NCG = NB // CGRP
                    for b in range(NB):
                        if b % CGRP == 0:
                            stage_c_prod(b // CGRP)
                        b_matmuls(b, psT)
                        b_copyout(b, psT)
                        if b % CGRP == CGRP - 1:
                            stage_c(b // CGRP, psC, psO)

            nc.sync.dma_start(out=out_d[:, :], in_=out_row[0:1, :])

    return _split_multi_waits(nc) if split_waits else nc


def _col_shuffle_perm():
    """d[m]: DRAM column position for original node m so the cnT transpose
    pairing (k = 2p + j) matches h8's block-major slots (node 128T + p at
    chunk T//2, j = T%2)."""
    m = np.arange(N)
    T = m // P
    p = m % P
    return 256 * (T // 2) + 2 * p + (T % 2)


def kernel(**inputs):
    from concourse.bass_utils import run_bass_kernel_spmd

    if "nc" not in _CACHE:
        _CACHE["nc"] = _build()
    nc = _CACHE["nc"]

    x = np.ascontiguousarray(inputs["x"], dtype=np.float32)
    adj8 = np.ascontiguousarray(inputs["adj"]).astype(ml_dtypes.float8_e4m3)
    d = _col_shuffle_perm()
    adjs = np.empty_like(adj8)
    adjs[:, d] = adj8
    tar = np.asarray(inputs["tar_ei"]).astype(np.int32)

    x8 = x.astype(ml_dtypes.float8_e4m3)
    # xa8[p, k*N + n] = x[n, k*128 + p]
    xa8 = np.ascontiguousarray(
        x8.T.reshape(2, P, N).transpose(1, 0, 2).reshape(P, 2 * N))
    # xr8t[p, T*C + c] = x[T*128 + p, c]
    xr8t = np.ascontiguousarray(
        x8.reshape(NT, P, C).transpose(1, 0, 2).reshape(P, NT * C))

    def wtile(w, dt):
        # [p, (ksub 2, cout C)] from [C, C]
        return np.ascontiguousarray(
            np.asarray(w).astype(dt).reshape(2, P, C).transpose(1, 0, 2)
            .reshape(P, 2 * C))

    wa8 = np.concatenate(
        [wtile(inputs["xlin_w1"], ml_dtypes.float8_e4m3),
         wtile(inputs["xlin_w2"], ml_dtypes.float8_e4m3)], axis=1)
    onesb2 = np.concatenate(
        [np.ones((1, P), np.float32),
         np.asarray(inputs["xlin_b2"], np.float32).reshape(1, C)],
        axis=1).astype(ml_dtypes.float8_e4m3)
    beta_v = float(np.asarray(inputs["beta"]).reshape(-1)[0])
    winp = {n: np.asarray(inputs[n], np.float32) for n in
            ("xcn_w1", "xcn_w2", "xij_w", "lin_w1")}
    winp["xcn_w2"] = winp["xcn_w2"] * beta_v
    wc = np.concatenate(
        [wtile(winp[n], ml_dtypes.bfloat16)
         for n in ("xcn_w1", "xcn_w2", "xij_w", "lin_w1")], axis=1)

    def btile(b):
        return np.ascontiguousarray(
            np.asarray(b, dtype=np.float32).reshape(2, P).T)

    binp = {n: np.asarray(inputs[n], np.float32) for n in
            ("xlin_b1", "xcn_b1", "xcn_b2", "xij_b", "lin_b1")}
    binp["xcn_b2"] = binp["xcn_b2"] * beta_v
    fpk = np.concatenate(
        [btile(binp[n]) for n in
         ("xlin_b1", "xcn_b1", "xcn_b2", "xij_b", "lin_b1")] +
        [np.full((P, 1), np.asarray(inputs["beta"]).reshape(-1)[0],
                 dtype=np.float32),
         np.full((P, 1), np.asarray(inputs["lin_b2"]).reshape(-1)[0],
                 dtype=np.float32)],
        axis=1)

    common = {
        "x": x.astype(ml_dtypes.bfloat16),
        "xa8": xa8,
        "xr8t": xr8t,
        "adjs": adjs,
        "wa8": wa8,
        "onesb2": onesb2,
        "wc": wc,
        "fpk": fpk,
        "lin_w2": np.ascontiguousarray(inputs["lin_w2"]).astype(
            ml_dtypes.bfloat16),
    }

    in_maps = []
    for c in range(NCORES):
        m = dict(common)
        m["idx"] = np.ascontiguousarray(tar[:, c * EL:(c + 1) * EL])
        in_maps.append(m)

    res = run_bass_kernel_spmd(
        nc, in_maps, core_ids=list(range(NCORES)), trace=TRACE
    )
    global LAST_RESULT
    LAST_RESULT = res
    out = np.concatenate(
        [res.results[c]["out"].reshape(EL, 1) for c in range(NCORES)], axis=0
    )
    return out.astype(np.float32)



# revision 12
# speedup vs baseline: 1.5121x; 1.5121x over previous
"""CNLinkPredictor Trainium2 kernel, v2 (fused gather-transpose pipeline).

Edge-sharded across 8 NeuronCores (1024 target edges each); x, adj, and the
MLP weights are replicated. Per core:

  A) h = x + MLP(x) with BOTH layers in fp8 DoubleRow:
       - L1: stationary w1 [p][ksub stride 256][cout], moving xa8i
         byte-interleaved pairs (host: xa8i[p, 2n+j] = x[n, 128j+p]).
       - L2 flips orientation: stationary y1T pairs (ch halves, pair step
         512), moving w2i interleaved (host: w2i[p, 2c+j] = W2[128j+p, c]);
         bias lands via a K=1 ones-row x (b2,b2)-row matmul; relu
         (Act even / DVE odd halves) writes fp8 straight into h8.
       - the x residual is folded into a gpsimd CCE DMA: h8 += xr8t
         (DRAM->SBUF accumulate), one [128, 4096] copy per 4 groups.
  B) per 256-edge group: dma_gather(transpose=True) pulls 256 adjacency
     rows per endpoint ALREADY node-partitioned (the host column shuffle
     makes the 16-bit-granularity transpose line up with h8's DoubleRow
     block slots), one u16 bitwise-AND per group, then 64 DoubleRow fp8
     matmuls accumulating xcnT[c, e] directly.  No separate xbar
     transposes, no per-row indirect DMAs (SWDGE fixed cost ~1us/instr).
  C) edge MLPs in transposed layout (bf16) per 256-edge group: xiT/xjT
     come from two more transposing gathers, beta folded into
     xcn_w2/xcn_b2 on the host, z = u2 + xij never materializes (lin_w1
     distributes over the sum in PSUM).

Scheduling notes:
  - Pool (gpsimd) queue order: g0 gathers, xiT/xjT, g1, then cce0/g2,
    cce1/g3, cce2, cce3 interleaved so CCE adds never head-of-line-block
    the adjacency gathers that feed the critical cn pipeline.
  - SP queue: consts + xa8i chunks first, idx16 LAST (it gates the Pool
    gathers, keeping stage-A feeds ahead of them on the DMA engines);
    wc rides the Act queue after g0's relus.
  - DVE queue: stage-A L2 relus, prod, AND(G0..3), C3 acts - every op's
    input is ready by the time the in-order stream reaches it.
  - Act queue: L1 relus, wc load, then per group copyout + stage-C acts.

Hardware pitfalls carried from v1 (all still honored):
  - walrus accepts at most ONE sync-wait per instruction
    (_apply_tile_patch + _split_multi_waits).
  - 4-byte DMA traffic corrupts in-flight 2-byte xbar transposes: the
    f32 fpk load happens before any transposing gather is in flight, the
    single f32 store happens last; everything in between is <= 2 B/elem.
  - DoubleRow stationary must be block-major (pair step % 16 == 0):
    w1 step 256, y1T step 512, h8 step 256; byte-interleaved layouts
    (xa8i, w2i, cnT) only ever appear as the MOVING operand.
  - PSUM zero regions are 2048 B; every accumulation psum tile occupies
    a full bank, so start=True zeroing never clobbers a neighbor.
  - GPSIMD cannot access PSUM - PSUM->SBUF copies stay on Act/DVE.
"""

import numpy as np
import ml_dtypes

N = 8192
C = 256
E = 8192
NCORES = 8
EL = E // NCORES          # edges per core
P = 128
NCHUNK = N // 256         # 256-node DoubleRow chunks (32)
NT = N // P               # node tiles (64)
AGRP = 512                # stage-A node group (4 tiles)
NG = N // AGRP            # stage-A groups (16)
EG = 256                  # edges per gather group
NEG = EL // EG            # gather groups per core (4)

_CACHE = {}
TRACE = False
LAST_RESULT = None


def _apply_tile_patch():
    """Split the Tile tail-drain's multi-sem wait onto individual SP nops."""
    from concourse.tile import TileContext
    from concourse.vector_clock import ScopedClock

    if getattr(TileContext, "_drain_patched", False):
        return

    def _patched(self, tick_clock, wait_clock):
        nc = self.nc
        collector = nc.sync.nop()
        wait_clock.add_sem_waits(
            collector.ins, ScopedClock({None: tick_clock.global_clock})
        )
        si = collector.ins.sync_info
        waits = list(si.on_wait) if si is not None and si.on_wait else []
        if si is not None and len(waits) > 1:
            name_to_handle = {h.name: h for h in self.sems.allocated().values()}
            si.on_wait = [waits[0]]
            for w in waits[1:]:
                op = {
                    "sem-ge-imm": "sem-ge",
                    "sem-eq-imm": "sem-eq",
                    "sem-le-imm": "sem-le",
                }.get(str(w.wait_mode), "sem-ge")
                nc.sync.nop().wait_op(name_to_handle[w.ant_name], w.wait_value, op)
        nc.sync.drain()
        nc.all_engine_barrier()
        assert self.sems is not None
        popped = nc._tile_sem_poison_stack.pop()
        assert popped is self._sem_poison
        nc.clear_and_free_semaphores(list(self.sems.allocated().values()))
        nc.all_engine_barrier()

    TileContext._drain_and_barrier = _patched
    TileContext._drain_patched = True


def _split_multi_waits(nc):
    """Hoist extra sync-waits onto same-engine NoOps (sequential waits ==
    ANDed waits); this walrus build allows one wait per instruction."""
    import concourse.mybir as mybir

    cnt = 0
    for fn in nc.m.functions:
        for bb in fn.blocks:
            out = []
            for inst in bb.instructions:
                si = getattr(inst, "sync_info", None)
                waits = list(si.on_wait) if si is not None and si.on_wait else []
                if len(waits) > 1:
                    for w in waits[:-1]:
                        nop = mybir.InstNoOp(name=f"ws-{cnt}", ins=[], outs=[])
                        cnt += 1
                        nop.engine = inst.engine
                        nop.sync_info = mybir.SyncInfo(on_wait=[w], on_update=[])
                        out.append(nop)
                    si.on_wait = [waits[-1]]
                out.append(inst)
            bb.instructions = out
    return nc


def _build(split_waits=True):
    import concourse.bass as bass
    import concourse.mybir as mybir
    from concourse.tile import TileContext

    _apply_tile_patch()

    f32 = mybir.dt.float32
    bf16 = mybir.dt.bfloat16
    fp8 = mybir.dt.float8e4
    u16 = mybir.dt.uint16
    i16 = mybir.dt.int16
    Relu = mybir.ActivationFunctionType.Relu
    Ident = mybir.ActivationFunctionType.Identity
    MUL = mybir.AluOpType.mult
    ADD = mybir.AluOpType.add
    AND = mybir.AluOpType.bitwise_and
    DR = mybir.MatmulPerfMode.DoubleRow

    nc = bass.Bass(num_swdge_queues=4, dynamic_dma_scratch_size=32768)

    # host-pretiled: xa8i[p, 2n+j] = x[n, 128j+p] (fp8, DR moving pairs)
    xa8i_d = nc.dram_tensor("xa8i", [P, 2 * N], fp8, kind="ExternalInput")
    # host-pretiled: xr8t[p, T*C + c] = x[T*128 + p, c] (fp8, h8 layout)
    xr8_d = nc.dram_tensor("xr8t", [P, 2 * N], fp8, kind="ExternalInput")
    x_d = nc.dram_tensor("x", [N, C], bf16, kind="ExternalInput")
    adjs_d = nc.dram_tensor("adjs", [N, N], fp8, kind="ExternalInput")
    # wrapped i16 gather indices: idx16[16k+p, which*64 + s] =
    # tar[which, 16s+p] (16-partition wrap replicated for the 8 Q7 cores)
    idx16_d = nc.dram_tensor("idx16", [P, 2 * EL // 16], i16,
                             kind="ExternalInput")
    # fp8 stage-A weights: w1 [p,(ksub 2,cout 256)] ++ w2i [p, 2c+j]
    wa8_d = nc.dram_tensor("wa8", [P, 4 * C], fp8, kind="ExternalInput")
    onesb2_d = nc.dram_tensor("onesb2", [1, P + 2 * C], fp8,
                              kind="ExternalInput")
    # bf16 stage-C weights: [p, (which 4, k 2, cout 256)]
    wc_d = nc.dram_tensor("wc", [P, 8 * C], bf16, kind="ExternalInput")
    lin_w2_d = nc.dram_tensor("lin_w2", [C, 1], bf16, kind="ExternalInput")
    bnames = ["xlin_b1", "xcn_b1", "xcn_b2", "xij_b", "lin_b1"]
    fpk_d = nc.dram_tensor("fpk", [P, 2 * len(bnames) + 2], f32,
                           kind="ExternalInput")
    out_d = nc.dram_tensor("out", [1, EL], f32, kind="ExternalOutput")

    _gq = [0]

    def _gqn():
        q = _gq[0] % 4
        _gq[0] += 1
        return q

    from concourse import library_config

    with TileContext(nc) as tc:
        # dma_gather lives in the 'mlp' gpsimd library; load it before any
        # Pool-queue gather dispatches.
        nc.gpsimd.load_library(library_config.mlp)
        with (
            tc.tile_pool(name="const", bufs=1) as pK,
            tc.tile_pool(name="h8p", bufs=1) as pH,
            tc.tile_pool(name="adj", bufs=2) as pAdj,
            tc.tile_pool(name="cn", bufs=2) as pCn,
            tc.tile_pool(name="xcnT", bufs=1) as pXT,
            tc.tile_pool(name="xij", bufs=1) as pXi,
            tc.tile_pool(name="prod", bufs=1) as pPr,
            tc.tile_pool(name="edge", bufs=1) as pC,
        ):
            # ---- constants (SP queue; f32 fpk first, idx16 LAST so the
            # Pool gathers start only after the stage-A feeds are queued) ----
            fpk = pK.tile([P, 2 * len(bnames) + 2], f32, tag="fpk",
                          name="fpk")
            nc.sync.dma_start(out=fpk[:], in_=fpk_d[:, :])
            b_sb = {}
            for q, n in enumerate(bnames):
                b_sb[n] = fpk[:, 2 * q:2 * q + 2]
            lb2_sb = fpk[:, 11:12]

            wa8 = pK.tile([P, 4 * C], fp8, tag="wa8", name="wa8")
            nc.sync.dma_start(out=wa8[:], in_=wa8_d[:, :])
            w1_v = wa8[:, 0:2 * C].rearrange("p (j m) -> p j m", j=2)
            w2i_v = wa8[:, 2 * C:4 * C].rearrange("p (c j) -> p j c", j=2)
            onesb2 = pK.tile([1, P + 2 * C], fp8, tag="onesb2", name="onesb2")
            nc.sync.dma_start(out=onesb2[:], in_=onesb2_d[:, :])
            ones_sb = onesb2[:, 0:P]
            b2row2_sb = onesb2[:, P:P + 2 * C]

            lw2_t = pK.tile([P, 2], bf16, tag="lin_w2", name="lin_w2t")
            nc.sync.dma_start(
                out=lw2_t[:].rearrange("p (k o) -> p k o", k=2),
                in_=lin_w2_d[:, :].rearrange("(k p) o -> p k o", p=P),
            )
            lw2_sb = [lw2_t[:, 0:1], lw2_t[:, 1:2]]

            xa8i = pK.tile([P, 2 * N], fp8, tag="xa8i", name="xa8i")
            xr8sb = pK.tile([P, 2 * N], fp8, tag="xr8sb", name="xr8sb")
            for ck in range(4):
                sl = slice(ck * (N // 2), (ck + 1) * (N // 2))
                nc.sync.dma_start(out=xa8i[:, sl], in_=xa8i_d[:, sl])
                nc.sync.dma_start(out=xr8sb[:, sl], in_=xr8_d[:, sl])
            xa8i_v = xa8i[:].rearrange("p (n j) -> p j n", j=2)

            idx16 = pK.tile([P, 2 * EL // 16], i16, tag="idx16",
                            name="idx16")
            nc.sync.dma_start(out=idx16[:], in_=idx16_d[:, :])

            # wc is loaded later on the Act queue (after g0's relus) so its
            # transfer lands behind the stage-A feeds and first gathers.
            wc_t = pK.tile([P, 8 * C], bf16, tag="wc", name="wc")
            wC_sb = {}
            for q, n in enumerate(("xcn_w1", "xcn_w2", "xij_w", "lin_w1")):
                wC_sb[n] = [wc_t[:, q * 2 * C:q * 2 * C + C],
                            wc_t[:, q * 2 * C + C:(q + 1) * 2 * C]]

            out_row = pK.tile([1, EL], f32, tag="out_row", name="out_row")

            # h8[p, T*256 + c] = h[node 128*T + p, channel c] in fp8.
            h8 = pH.tile([P, 2 * N], fp8, tag="h8", name="h8")
            h8_v = h8[:].rearrange(
                "p (ck j ch c2) -> p ck ch j c2", ck=NCHUNK, j=2, ch=2)

            # ---- transposing gathers (Pool queue) ----
            def adj_gather(which, G):
                t = pAdj.tile([P, 2 * N], fp8, tag=f"a{which}",
                              name=f"a{which}{G}")
                w = 0 if which == "i" else 1
                nc.gpsimd.dma_gather(
                    t[:].rearrange("p (a b) -> p a b", a=64),
                    adjs_d[:, :],
                    idx16[:, w * 64 + G * 16:w * 64 + (G + 1) * 16],
                    EG, EG, N,
                    transpose=True,
                    queue_num=_gqn(),
                )
                return t

            def x_gather(which):
                # transposing gathers crash the Q7 ucode above 512 idxs
                # (observed empirically: 512 exact, 1024 wedges the device),
                # so the EL=1024 edge gather is split into two halves.
                # layout [p, (half 2, a 2, e 512)]: edge e = 512*half + e'
                t = pXi.tile([P, 2 * EL], bf16, tag=f"x{which}",
                             name=f"x{which}T")
                w = 0 if which == "i" else 1
                for h in range(2):
                    nc.gpsimd.dma_gather(
                        t[:, h * EL:(h + 1) * EL].rearrange(
                            "p (a b) -> p a b", a=2),
                        x_d[:, :],
                        idx16[:, w * 64 + h * 32:w * 64 + (h + 1) * 32],
                        EL // 2, EL // 2, C,
                        transpose=True,
                        queue_num=_gqn(),
                    )
                return t

            gath = {}
            gath[0] = (adj_gather("i", 0), adj_gather("j", 0))
            xiT = x_gather("i")
            xjT = x_gather("j")
            gath[1] = (adj_gather("i", 1), adj_gather("j", 1))
            gath[2] = (adj_gather("i", 2), adj_gather("j", 2))
            gath[3] = (adj_gather("i", 3), adj_gather("j", 3))

            # ---- stage A ----
            with tc.tile_pool(name="stA", bufs=3) as pA, \
                 tc.tile_pool(name="psA", bufs=4, space="PSUM") as psA, \
                 tc.tile_pool(name="psL2", bufs=4, space="PSUM") as psL2:
                for g in range(NG):
                    m0 = g * AGRP
                    y1T = pA.tile([P, 2 * AGRP], fp8, tag="y1T",
                                  name=f"y1T{g}")
                    for ch in range(2):
                        ps = psA.tile([P, AGRP], f32, tag="psA",
                                      name=f"psA_{g}{ch}")
                        nc.tensor.matmul(
                            ps[:], w1_v[:, :, ch * P:(ch + 1) * P],
                            xa8i_v[:, :, m0:m0 + AGRP],
                            start=True, stop=True, perf_mode=DR,
                        )
                        nc.scalar.activation(
                            y1T[:, ch * AGRP:(ch + 1) * AGRP], ps[:], Relu,
                            bias=b_sb["xlin_b1"][:, ch:ch + 1],
                        )
                    y1_v = y1T[:].rearrange("p (j n) -> p j n", j=2)
                    for half in range(2):
                        ps2 = psL2.tile([P, 2 * C], f32, tag="psL2",
                                        name=f"psL2_{g}{half}")
                        for tt in range(2):
                            t2 = 2 * half + tt
                            nc.tensor.matmul(
                                ps2[:, tt * C:(tt + 1) * C],
                                y1_v[:, :, t2 * P:(t2 + 1) * P],
                                w2i_v[:, :, :],
                                start=(tt == 0), stop=False, perf_mode=DR,
                            )
                        nc.tensor.matmul(
                            ps2[:], ones_sb[0:1, :], b2row2_sb[0:1, :],
                            start=False, stop=True,
                        )
                        c0 = (4 * g + 2 * half) * C
                        # fused relu + residual: h8 = max(ps2, 0) + x8
                        nc.vector.scalar_tensor_tensor(
                            h8[:, c0:c0 + 2 * C], ps2[:], 0.0,
                            xr8sb[:, c0:c0 + 2 * C],
                            mybir.AluOpType.max, ADD)
                    if g == 0:
                        nc.scalar.dma_start(out=wc_t[:], in_=wc_d[:, :])

            # ---- stages B + C ----
            xcnT_sb = [
                pXT.tile([P, EL], bf16, tag=f"xcnT{ch}", name=f"xcnT{ch}")
                for ch in range(2)
            ]
            prodT = pPr.tile([P, 2 * EL], bf16, tag="prodT", name="prodT")
            nc.vector.tensor_tensor(
                out=prodT[:], in0=xiT[:], in1=xjT[:], op=MUL)

            with tc.tile_pool(name="psB", bufs=4, space="PSUM") as psB, \
                 tc.tile_pool(name="psC", bufs=3, space="PSUM") as psC, \
                 tc.tile_pool(name="psO", bufs=1, space="PSUM") as psO:

                def stage_c(G):
                    W = EG
                    on_dve = (G == NEG - 1)

                    def act(t, ps, bname, h):
                        if on_dve:
                            nc.vector.tensor_scalar(
                                t[:], ps[:], b_sb[bname][:, h:h + 1], 0.0,
                                ADD, mybir.AluOpType.max)
                        else:
                            nc.scalar.activation(
                                t[:], ps[:], Relu,
                                bias=b_sb[bname][:, h:h + 1])

                    def mlp_layer(r0, r1, wname, bname, outtag):
                        outs = []
                        for h in range(2):
                            ps = psC.tile([P, W], f32, tag="psc",
                                          name=f"psc_{G}_{outtag}{h}")
                            nc.tensor.matmul(
                                ps[:], wC_sb[wname][0][:, h * P:(h + 1) * P],
                                r0, start=True, stop=False,
                            )
                            nc.tensor.matmul(
                                ps[:], wC_sb[wname][1][:, h * P:(h + 1) * P],
                                r1, start=False, stop=True,
                            )
                            t = pC.tile([P, W], bf16, tag=f"{outtag}{h}",
                                        name=f"{outtag}{h}_{G}")
                            act(t, ps, bname, h)
                            outs.append(t)
                        return outs

                    sl = slice(G * W, (G + 1) * W)
                    # prodT layout [p, (half 2, a 2, e 512)]
                    pr0 = (G // 2) * EL + (G % 2) * W
                    xijT = mlp_layer(prodT[:, pr0:pr0 + W],
                                     prodT[:, pr0 + EL // 2:pr0 + EL // 2 + W],
                                     "xij_w", "xij_b", "xijT")
                    u1T = mlp_layer(xcnT_sb[0][:, sl], xcnT_sb[1][:, sl],
                                    "xcn_w1", "xcn_b1", "u1T")
                    u2T = mlp_layer(u1T[0][:], u1T[1][:],
                                    "xcn_w2", "xcn_b2", "u2T")
                    vT = []
                    for h in range(2):
                        ps = psC.tile([P, W], f32, tag="psc",
                                      name=f"psc_{G}_vT{h}")
                        nc.tensor.matmul(
                            ps[:], wC_sb["lin_w1"][0][:, h * P:(h + 1) * P],
                            u2T[0][:], start=True, stop=False,
                        )
                        nc.tensor.matmul(
                            ps[:], wC_sb["lin_w1"][1][:, h * P:(h + 1) * P],
                            u2T[1][:], start=False, stop=False,
                        )
                        nc.tensor.matmul(
                            ps[:], wC_sb["lin_w1"][0][:, h * P:(h + 1) * P],
                            xijT[0][:], start=False, stop=False,
                        )
                        nc.tensor.matmul(
                            ps[:], wC_sb["lin_w1"][1][:, h * P:(h + 1) * P],
                            xijT[1][:], start=False, stop=True,
                        )
                        t = pC.tile([P, W], bf16, tag=f"vT{h}",
                                    name=f"vT{h}_{G}")
                        act(t, ps, "lin_b1", h)
                        vT.append(t)
                    pso = psO.tile([1, W], f32, tag="pso", name=f"pso{G}")
                    nc.tensor.matmul(
                        pso[:], lw2_sb[0][:], vT[0][:], start=True,
                        stop=False)
                    nc.tensor.matmul(
                        pso[:], lw2_sb[1][:], vT[1][:], start=False,
                        stop=True)
                    nc.scalar.activation(
                        out_row[0:1, G * W:(G + 1) * W], pso[:],
                        Ident, bias=lb2_sb[0:1, 0:1],
                    )

                for G in range(NEG):
                    ai, aj = gath[G]
                    cn = pCn.tile([P, 2 * N], fp8, tag="cn", name=f"cn{G}")
                    nc.vector.tensor_tensor(
                        out=cn[:].bitcast(u16),
                        in0=ai[:].bitcast(u16),
                        in1=aj[:].bitcast(u16),
                        op=AND,
                    )
                    cn_v = cn[:].rearrange(
                        "p (ck e j) -> p ck j e", ck=NCHUNK, j=2)
                    for ch in range(2):
                        psb = psB.tile([P, EG], f32, tag="psb",
                                       name=f"psb_{G}{ch}")
                        for ck in range(NCHUNK):
                            nc.tensor.matmul(
                                psb[:], h8_v[:, ck, ch, :, :],
                                cn_v[:, ck, :, :],
                                start=(ck == 0), stop=(ck == NCHUNK - 1),
                                perf_mode=DR,
                            )
                        if G == NEG - 1:
                            nc.vector.tensor_copy(
                                xcnT_sb[ch][:, G * EG:(G + 1) * EG], psb[:])
                        else:
                            nc.scalar.activation(
                                xcnT_sb[ch][:, G * EG:(G + 1) * EG], psb[:],
                                Ident)
                    stage_c(G)

            nc.sync.dma_start(out=out_d[:, :], in_=out_row[0:1, :])

    # Populate .instr bytes for extended-inst InstISA subclasses (the
    # PseudoReloadLibraryIndex library load + DMAGatherAnt). Raw Bass does
    # not run Bacc's codegen pass; without this walrus sees empty .instr
    # and fails codegen with "ISA wrong length".
    mybir.codegen_inst_isa_subclasses(nc)
    return _split_multi_waits(nc) if split_waits else nc


def _col_shuffle_perm():
    """d[m]: DRAM column position for original node m so the 16-bit
    transposing gather lands bytes exactly in the DoubleRow moving layout
    (node 256*ck + 128*j + p at [p, ck, byte 2e+j])."""
    m = np.arange(N)
    T = m // P
    p = m % P
    return 256 * (T // 2) + 2 * p + (T % 2)


def kernel(**inputs):
    from concourse.bass_utils import run_bass_kernel_spmd

    if "nc" not in _CACHE:
        _CACHE["nc"] = _build()
    nc = _CACHE["nc"]

    x = np.ascontiguousarray(inputs["x"], dtype=np.float32)
    adj8 = np.ascontiguousarray(inputs["adj"]).astype(ml_dtypes.float8_e4m3)
    d = _col_shuffle_perm()
    adjs = np.empty_like(adj8)
    adjs[:, d] = adj8
    tar = np.asarray(inputs["tar_ei"]).astype(np.int16)

    x8 = x.astype(ml_dtypes.float8_e4m3)
    # xa8i[p, 2n+j] = x[n, 128j+p]
    xa8i = np.ascontiguousarray(
        x8.reshape(N, 2, P).transpose(2, 0, 1).reshape(P, 2 * N))
    # xr8t[p, T*C + c] = x[T*128 + p, c]
    xr8t = np.ascontiguousarray(
        x8.reshape(NT, P, C).transpose(1, 0, 2).reshape(P, NT * C))

    def wtile(w, dt):
        # [p, (ksub 2, cout C)] from [C, C]
        return np.ascontiguousarray(
            np.asarray(w).astype(dt).reshape(2, P, C).transpose(1, 0, 2)
            .reshape(P, 2 * C))

    w2_8 = np.asarray(inputs["xlin_w2"]).astype(ml_dtypes.float8_e4m3)
    # w2i[p, 2c+j] = W2[128j+p, c]
    w2i = np.ascontiguousarray(
        w2_8.reshape(2, P, C).transpose(1, 2, 0).reshape(P, 2 * C))
    wa8 = np.concatenate(
        [wtile(inputs["xlin_w1"], ml_dtypes.float8_e4m3), w2i], axis=1)
    b2 = np.asarray(inputs["xlin_b2"], np.float32).reshape(1, C)
    onesb2 = np.concatenate(
        [np.ones((1, P), np.float32), b2, b2],
        axis=1).astype(ml_dtypes.float8_e4m3)
    beta_v = float(np.asarray(inputs["beta"]).reshape(-1)[0])
    winp = {n: np.asarray(inputs[n], np.float32) for n in
            ("xcn_w1", "xcn_w2", "xij_w", "lin_w1")}
    winp["xcn_w2"] = winp["xcn_w2"] * beta_v
    wc = np.concatenate(
        [wtile(winp[n], ml_dtypes.bfloat16)
         for n in ("xcn_w1", "xcn_w2", "xij_w", "lin_w1")], axis=1)

    def btile(b):
        return np.ascontiguousarray(
            np.asarray(b, dtype=np.float32).reshape(2, P).T)

    binp = {n: np.asarray(inputs[n], np.float32) for n in
            ("xlin_b1", "xcn_b1", "xcn_b2", "xij_b", "lin_b1")}
    binp["xcn_b2"] = binp["xcn_b2"] * beta_v
    fpk = np.concatenate(
        [btile(binp[n]) for n in
         ("xlin_b1", "xcn_b1", "xcn_b2", "xij_b", "lin_b1")] +
        [np.full((P, 1), beta_v, dtype=np.float32),
         np.full((P, 1), np.asarray(inputs["lin_b2"]).reshape(-1)[0],
                 dtype=np.float32)],
        axis=1)

    common = {
        "x": x.astype(ml_dtypes.bfloat16),
        "xa8i": xa8i,
        "xr8t": xr8t,
        "adjs": adjs,
        "wa8": wa8,
        "onesb2": onesb2,
        "wc": wc,
        "fpk": fpk,
        "lin_w2": np.ascontiguousarray(inputs["lin_w2"]).astype(
            ml_dtypes.bfloat16),
    }

    in_maps = []
    for c in range(NCORES):
        m = dict(common)
        tc_ = tar[:, c * EL:(c + 1) * EL]  # [2, EL]
        idx16 = np.empty((16, 2 * EL // 16), np.int16)
        for w in range(2):
            idx16[:, w * 64:(w + 1) * 64] = tc_[w].reshape(64, 16).T
        m["idx16"] = np.ascontiguousarray(np.tile(idx16, (8, 1)))
        in_maps.append(m)

    res = run_bass_kernel_spmd(
        nc, in_maps, core_ids=list(range(NCORES)), trace=TRACE
    )
    global LAST_RESULT
    LAST_RESULT = res
    out = np.concatenate(
        [res.results[c]["out"].reshape(EL, 1) for c in range(NCORES)], axis=0
    )
    return out.astype(np.float32)


# revision 13
# speedup vs baseline: 1.7436x; 1.1531x over previous
"""CNLinkPredictor Trainium2 kernel, v2 (fused gather-transpose pipeline).

Edge-sharded across 8 NeuronCores (1024 target edges each); x, adj, and the
MLP weights are replicated. Per core:

  A) h = x + MLP(x) with BOTH layers in fp8 DoubleRow:
       - L1: stationary w1 [p][ksub stride 256][cout], moving xa8i
         byte-interleaved pairs (host: xa8i[p, 2n+j] = x[n, 128j+p]).
       - L2 flips orientation: stationary y1T pairs (ch halves, pair step
         512), moving w2i interleaved (host: w2i[p, 2c+j] = W2[128j+p, c]);
         bias lands via a K=1 ones-row x (b2,b2)-row matmul; relu
         (Act even / DVE odd halves) writes fp8 straight into h8.
       - the x residual is folded into a gpsimd CCE DMA: h8 += xr8t
         (DRAM->SBUF accumulate), one [128, 4096] copy per 4 groups.
  B) per 256-edge group: dma_gather(transpose=True) pulls 256 adjacency
     rows per endpoint ALREADY node-partitioned (the host column shuffle
     makes the 16-bit-granularity transpose line up with h8's DoubleRow
     block slots), one u16 bitwise-AND per group, then 64 DoubleRow fp8
     matmuls accumulating xcnT[c, e] directly.  No separate xbar
     transposes, no per-row indirect DMAs (SWDGE fixed cost ~1us/instr).
  C) edge MLPs in transposed layout (bf16) per 256-edge group: xiT/xjT
     come from two more transposing gathers, beta folded into
     xcn_w2/xcn_b2 on the host, z = u2 + xij never materializes (lin_w1
     distributes over the sum in PSUM).

Scheduling notes:
  - Pool (gpsimd) queue order: g0 gathers, xiT/xjT, g1, then cce0/g2,
    cce1/g3, cce2, cce3 interleaved so CCE adds never head-of-line-block
    the adjacency gathers that feed the critical cn pipeline.
  - SP queue: consts + xa8i chunks first, idx16 LAST (it gates the Pool
    gathers, keeping stage-A feeds ahead of them on the DMA engines);
    wc rides the Act queue after g0's relus.
  - DVE queue: stage-A L2 relus, prod, AND(G0..3), C3 acts - every op's
    input is ready by the time the in-order stream reaches it.
  - Act queue: L1 relus, wc load, then per group copyout + stage-C acts.

Hardware pitfalls carried from v1 (all still honored):
  - walrus accepts at most ONE sync-wait per instruction
    (_apply_tile_patch + _split_multi_waits).
  - 4-byte DMA traffic corrupts in-flight 2-byte xbar transposes: the
    f32 fpk load happens before any transposing gather is in flight, the
    single f32 store happens last; everything in between is <= 2 B/elem.
  - DoubleRow stationary must be block-major (pair step % 16 == 0):
    w1 step 256, y1T step 512, h8 step 256; byte-interleaved layouts
    (xa8i, w2i, cnT) only ever appear as the MOVING operand.
  - PSUM zero regions are 2048 B; every accumulation psum tile occupies
    a full bank, so start=True zeroing never clobbers a neighbor.
  - GPSIMD cannot access PSUM - PSUM->SBUF copies stay on Act/DVE.
"""

import numpy as np
import ml_dtypes

N = 8192
C = 256
E = 8192
NCORES = 8
EL = E // NCORES          # edges per core
P = 128
NCHUNK = N // 256         # 256-node DoubleRow chunks (32)
NT = N // P               # node tiles (64)
AGRP = 512                # stage-A node group (4 tiles)
NG = N // AGRP            # stage-A groups (16)
EG = 128                  # edges per gather group
NEG = EL // EG            # gather groups per core (8)

_CACHE = {}
TRACE = False
LAST_RESULT = None


def _apply_tile_patch():
    """Split the Tile tail-drain's multi-sem wait onto individual SP nops."""
    from concourse.tile import TileContext
    from concourse.vector_clock import ScopedClock

    if getattr(TileContext, "_drain_patched", False):
        return

    def _patched(self, tick_clock, wait_clock):
        nc = self.nc
        collector = nc.sync.nop()
        wait_clock.add_sem_waits(
            collector.ins, ScopedClock({None: tick_clock.global_clock})
        )
        si = collector.ins.sync_info
        waits = list(si.on_wait) if si is not None and si.on_wait else []
        if si is not None and len(waits) > 1:
            name_to_handle = {h.name: h for h in self.sems.allocated().values()}
            si.on_wait = [waits[0]]
            for w in waits[1:]:
                op = {
                    "sem-ge-imm": "sem-ge",
                    "sem-eq-imm": "sem-eq",
                    "sem-le-imm": "sem-le",
                }.get(str(w.wait_mode), "sem-ge")
                nc.sync.nop().wait_op(name_to_handle[w.ant_name], w.wait_value, op)
        nc.sync.drain()
        nc.all_engine_barrier()
        assert self.sems is not None
        popped = nc._tile_sem_poison_stack.pop()
        assert popped is self._sem_poison
        nc.clear_and_free_semaphores(list(self.sems.allocated().values()))
        nc.all_engine_barrier()

    TileContext._drain_and_barrier = _patched
    TileContext._drain_patched = True


def _split_multi_waits(nc):
    """Hoist extra sync-waits onto same-engine NoOps (sequential waits ==
    ANDed waits); this walrus build allows one wait per instruction."""
    import concourse.mybir as mybir

    cnt = 0
    for fn in nc.m.functions:
        for bb in fn.blocks:
            out = []
            for inst in bb.instructions:
                si = getattr(inst, "sync_info", None)
                waits = list(si.on_wait) if si is not None and si.on_wait else []
                if len(waits) > 1:
                    for w in waits[:-1]:
                        nop = mybir.InstNoOp(name=f"ws-{cnt}", ins=[], outs=[])
                        cnt += 1
                        nop.engine = inst.engine
                        nop.sync_info = mybir.SyncInfo(on_wait=[w], on_update=[])
                        out.append(nop)
                    si.on_wait = [waits[-1]]
                out.append(inst)
            bb.instructions = out
    return nc


def _build(split_waits=True):
    import concourse.bass as bass
    import concourse.mybir as mybir
    from concourse.tile import TileContext

    _apply_tile_patch()

    f32 = mybir.dt.float32
    bf16 = mybir.dt.bfloat16
    fp8 = mybir.dt.float8e4
    u16 = mybir.dt.uint16
    i16 = mybir.dt.int16
    Relu = mybir.ActivationFunctionType.Relu
    Ident = mybir.ActivationFunctionType.Identity
    MUL = mybir.AluOpType.mult
    ADD = mybir.AluOpType.add
    AND = mybir.AluOpType.bitwise_and
    DR = mybir.MatmulPerfMode.DoubleRow

    nc = bass.Bass(num_swdge_queues=4, dynamic_dma_scratch_size=32768)

    # host-pretiled: xa8i[p, 2n+j] = x[n, 128j+p] (fp8, DR moving pairs)
    xa8i_d = nc.dram_tensor("xa8i", [P, 2 * N], fp8, kind="ExternalInput")
    # host-pretiled: xr8t[p, T*C + c] = x[T*128 + p, c] (fp8, h8 layout)
    xr8_d = nc.dram_tensor("xr8t", [P, 2 * N], fp8, kind="ExternalInput")
    x_d = nc.dram_tensor("x", [N, C], bf16, kind="ExternalInput")
    adjs_d = nc.dram_tensor("adjs", [N, N], fp8, kind="ExternalInput")
    # wrapped i16 gather indices: idx16[16k+p, which*64 + s] =
    # tar[which, 16s+p] (16-partition wrap replicated for the 8 Q7 cores)
    idx16_d = nc.dram_tensor("idx16", [P, 2 * EL // 16], i16,
                             kind="ExternalInput")
    # fp8 stage-A weights: w1 [p,(ksub 2,cout 256)] ++ w2i [p, 2c+j]
    wa8_d = nc.dram_tensor("wa8", [P, 4 * C], fp8, kind="ExternalInput")
    onesb2_d = nc.dram_tensor("onesb2", [1, P + 2 * C], fp8,
                              kind="ExternalInput")
    # bf16 stage-C weights: [p, (which 4, k 2, cout 256)]
    wc_d = nc.dram_tensor("wc", [P, 8 * C], bf16, kind="ExternalInput")
    lin_w2_d = nc.dram_tensor("lin_w2", [C, 1], bf16, kind="ExternalInput")
    bnames = ["xlin_b1", "xcn_b1", "xcn_b2", "xij_b", "lin_b1"]
    fpk_d = nc.dram_tensor("fpk", [P, 2 * len(bnames) + 2], f32,
                           kind="ExternalInput")
    out_d = nc.dram_tensor("out", [1, EL], f32, kind="ExternalOutput")

    _gq = [0]

    def _gqn():
        q = _gq[0] % 4
        _gq[0] += 1
        return q

    from concourse import library_config

    with TileContext(nc) as tc:
        # dma_gather lives in the 'mlp' gpsimd library; load it before any
        # Pool-queue gather dispatches.
        nc.gpsimd.load_library(library_config.mlp)
        with (
            tc.tile_pool(name="const", bufs=1) as pK,
            tc.tile_pool(name="h8p", bufs=1) as pH,
            tc.tile_pool(name="adj", bufs=4) as pAdj,
            tc.tile_pool(name="cn", bufs=3) as pCn,
            tc.tile_pool(name="xcnT", bufs=1) as pXT,
            tc.tile_pool(name="xij", bufs=1) as pXi,
            tc.tile_pool(name="prod", bufs=1) as pPr,
            tc.tile_pool(name="edge", bufs=1) as pC,
        ):
            # ---- constants (SP queue; f32 fpk first, idx16 LAST so the
            # Pool gathers start only after the stage-A feeds are queued) ----
            fpk = pK.tile([P, 2 * len(bnames) + 2], f32, tag="fpk",
                          name="fpk")
            nc.sync.dma_start(out=fpk[:], in_=fpk_d[:, :])
            b_sb = {}
            for q, n in enumerate(bnames):
                b_sb[n] = fpk[:, 2 * q:2 * q + 2]
            lb2_sb = fpk[:, 11:12]

            wa8 = pK.tile([P, 4 * C], fp8, tag="wa8", name="wa8")
            nc.sync.dma_start(out=wa8[:], in_=wa8_d[:, :])
            w1_v = wa8[:, 0:2 * C].rearrange("p (j m) -> p j m", j=2)
            w2i_v = wa8[:, 2 * C:4 * C].rearrange("p (c j) -> p j c", j=2)
            onesb2 = pK.tile([1, P + 2 * C], fp8, tag="onesb2", name="onesb2")
            nc.sync.dma_start(out=onesb2[:], in_=onesb2_d[:, :])
            ones_sb = onesb2[:, 0:P]
            b2row2_sb = onesb2[:, P:P + 2 * C]

            lw2_t = pK.tile([P, 2], bf16, tag="lin_w2", name="lin_w2t")
            nc.sync.dma_start(
                out=lw2_t[:].rearrange("p (k o) -> p k o", k=2),
                in_=lin_w2_d[:, :].rearrange("(k p) o -> p k o", p=P),
            )
            lw2_sb = [lw2_t[:, 0:1], lw2_t[:, 1:2]]

            xa8i = pK.tile([P, 2 * N], fp8, tag="xa8i", name="xa8i")
            xr8sb = pK.tile([P, 2 * N], fp8, tag="xr8sb", name="xr8sb")
            idx16 = pK.tile([P, 2 * EL // 16], i16, tag="idx16",
                            name="idx16")
            for ck in range(4):
                sl = slice(ck * (N // 2), (ck + 1) * (N // 2))
                nc.sync.dma_start(out=xa8i[:, sl], in_=xa8i_d[:, sl])
                nc.sync.dma_start(out=xr8sb[:, sl], in_=xr8_d[:, sl])
                if ck == 0:
                    # gathers gate on idx16; placing it after the first
                    # chunk pair lets them interleave with the remaining
                    # stage-A feeds on the DMA engines.
                    nc.sync.dma_start(out=idx16[:], in_=idx16_d[:, :])
            xa8i_v = xa8i[:].rearrange("p (n j) -> p j n", j=2)

            # wc is loaded later on the Act queue (after g0's relus) so its
            # transfer lands behind the stage-A feeds and first gathers.
            wc_t = pK.tile([P, 8 * C], bf16, tag="wc", name="wc")
            wC_sb = {}
            for q, n in enumerate(("xcn_w1", "xcn_w2", "xij_w", "lin_w1")):
                wC_sb[n] = [wc_t[:, q * 2 * C:q * 2 * C + C],
                            wc_t[:, q * 2 * C + C:(q + 1) * 2 * C]]

            out_row = pK.tile([1, EL], f32, tag="out_row", name="out_row")

            # h8[p, T*256 + c] = h[node 128*T + p, channel c] in fp8.
            h8 = pH.tile([P, 2 * N], fp8, tag="h8", name="h8")
            h8_v = h8[:].rearrange(
                "p (ck j ch c2) -> p ck ch j c2", ck=NCHUNK, j=2, ch=2)

            # ---- transposing gathers (Pool queue) ----
            def adj_gather(which, G):
                t = pAdj.tile([P, N], fp8, tag=f"a{which}",
                              name=f"a{which}{G}")
                w = 0 if which == "i" else 1
                nc.gpsimd.dma_gather(
                    t[:].rearrange("p (a b) -> p a b", a=64),
                    adjs_d[:, :],
                    idx16[:, w * 64 + G * 8:w * 64 + (G + 1) * 8],
                    EG, EG, N,
                    transpose=True,
                    queue_num=_gqn(),
                )
                return t

            def x_gather(which):
                # transposing gathers crash the Q7 ucode above 512 idxs
                # (observed empirically: 512 exact, 1024 wedges the device),
                # so the EL=1024 edge gather is split into two halves.
                # layout [p, (half 2, a 2, e 512)]: edge e = 512*half + e'
                t = pXi.tile([P, 2 * EL], bf16, tag=f"x{which}",
                             name=f"x{which}T")
                w = 0 if which == "i" else 1
                for h in range(2):
                    nc.gpsimd.dma_gather(
                        t[:, h * EL:(h + 1) * EL].rearrange(
                            "p (a b) -> p a b", a=2),
                        x_d[:, :],
                        idx16[:, w * 64 + h * 32:w * 64 + (h + 1) * 32],
                        EL // 2, EL // 2, C,
                        transpose=True,
                        queue_num=_gqn(),
                    )
                return t

            gath = {}
            gath[0] = (adj_gather("i", 0), adj_gather("j", 0))
            gath[1] = (adj_gather("i", 1), adj_gather("j", 1))
            xiT = x_gather("i")
            xjT = x_gather("j")
            for G in range(2, NEG):
                gath[G] = (adj_gather("i", G), adj_gather("j", G))

            # ---- stage A ----
            with tc.tile_pool(name="stA", bufs=3) as pA, \
                 tc.tile_pool(name="psA", bufs=4, space="PSUM") as psA, \
                 tc.tile_pool(name="psL2", bufs=4, space="PSUM") as psL2:
                for g in range(NG):
                    m0 = g * AGRP
                    y1T = pA.tile([P, 2 * AGRP], fp8, tag="y1T",
                                  name=f"y1T{g}")
                    for ch in range(2):
                        ps = psA.tile([P, AGRP], f32, tag="psA",
                                      name=f"psA_{g}{ch}")
                        nc.tensor.matmul(
                            ps[:], w1_v[:, :, ch * P:(ch + 1) * P],
                            xa8i_v[:, :, m0:m0 + AGRP],
                            start=True, stop=True, perf_mode=DR,
                        )
                        nc.scalar.activation(
                            y1T[:, ch * AGRP:(ch + 1) * AGRP], ps[:], Relu,
                            bias=b_sb["xlin_b1"][:, ch:ch + 1],
                        )
                    y1_v = y1T[:].rearrange("p (j n) -> p j n", j=2)
                    for half in range(2):
                        ps2 = psL2.tile([P, 2 * C], f32, tag="psL2",
                                        name=f"psL2_{g}{half}")
                        for tt in range(2):
                            t2 = 2 * half + tt
                            nc.tensor.matmul(
                                ps2[:, tt * C:(tt + 1) * C],
                                y1_v[:, :, t2 * P:(t2 + 1) * P],
                                w2i_v[:, :, :],
                                start=(tt == 0), stop=False, perf_mode=DR,
                            )
                        nc.tensor.matmul(
                            ps2[:], ones_sb[0:1, :], b2row2_sb[0:1, :],
                            start=False, stop=True,
                        )
                        c0 = (4 * g + 2 * half) * C
                        # fused relu + residual: h8 = max(ps2, 0) + x8
                        nc.vector.scalar_tensor_tensor(
                            h8[:, c0:c0 + 2 * C], ps2[:], 0.0,
                            xr8sb[:, c0:c0 + 2 * C],
                            mybir.AluOpType.max, ADD)
                    if g == 0:
                        nc.scalar.dma_start(out=wc_t[:], in_=wc_d[:, :])

            # ---- stages B + C ----
            xcnT_sb = [
                pXT.tile([P, EL], bf16, tag=f"xcnT{ch}", name=f"xcnT{ch}")
                for ch in range(2)
            ]
            prodT = pPr.tile([P, 2 * EL], bf16, tag="prodT", name="prodT")
            nc.vector.tensor_tensor(
                out=prodT[:], in0=xiT[:], in1=xjT[:], op=MUL)

            with tc.tile_pool(name="psB", bufs=4, space="PSUM") as psB, \
                 tc.tile_pool(name="psC", bufs=3, space="PSUM") as psC, \
                 tc.tile_pool(name="psO", bufs=1, space="PSUM") as psO:

                def stage_c(G):
                    W = 2 * EG
                    on_dve = (G == NEG // 2 - 1)

                    def act(t, ps, bname, h):
                        if on_dve:
                            nc.vector.tensor_scalar(
                                t[:], ps[:], b_sb[bname][:, h:h + 1], 0.0,
                                ADD, mybir.AluOpType.max)
                        else:
                            nc.scalar.activation(
                                t[:], ps[:], Relu,
                                bias=b_sb[bname][:, h:h + 1])

                    def mlp_layer(r0, r1, wname, bname, outtag):
                        outs = []
                        for h in range(2):
                            ps = psC.tile([P, W], f32, tag="psc",
                                          name=f"psc_{G}_{outtag}{h}")
                            nc.tensor.matmul(
                                ps[:], wC_sb[wname][0][:, h * P:(h + 1) * P],
                                r0, start=True, stop=False,
                            )
                            nc.tensor.matmul(
                                ps[:], wC_sb[wname][1][:, h * P:(h + 1) * P],
                                r1, start=False, stop=True,
                            )
                            t = pC.tile([P, W], bf16, tag=f"{outtag}{h}",
                                        name=f"{outtag}{h}_{G}")
                            act(t, ps, bname, h)
                            outs.append(t)
                        return outs

                    sl = slice(G * W, (G + 1) * W)
                    # prodT layout [p, (half 2, a 2, e 512)]
                    pr0 = (G // 2) * EL + (G % 2) * W
                    xijT = mlp_layer(prodT[:, pr0:pr0 + W],
                                     prodT[:, pr0 + EL // 2:pr0 + EL // 2 + W],
                                     "xij_w", "xij_b", "xijT")
                    u1T = mlp_layer(xcnT_sb[0][:, sl], xcnT_sb[1][:, sl],
                                    "xcn_w1", "xcn_b1", "u1T")
                    u2T = mlp_layer(u1T[0][:], u1T[1][:],
                                    "xcn_w2", "xcn_b2", "u2T")
                    vT = []
                    for h in range(2):
                        ps = psC.tile([P, W], f32, tag="psc",
                                      name=f"psc_{G}_vT{h}")
                        nc.tensor.matmul(
                            ps[:], wC_sb["lin_w1"][0][:, h * P:(h + 1) * P],
                            u2T[0][:], start=True, stop=False,
                        )
                        nc.tensor.matmul(
                            ps[:], wC_sb["lin_w1"][1][:, h * P:(h + 1) * P],
                            u2T[1][:], start=False, stop=False,
                        )
                        nc.tensor.matmul(
                            ps[:], wC_sb["lin_w1"][0][:, h * P:(h + 1) * P],
                            xijT[0][:], start=False, stop=False,
                        )
                        nc.tensor.matmul(
                            ps[:], wC_sb["lin_w1"][1][:, h * P:(h + 1) * P],
                            xijT[1][:], start=False, stop=True,
                        )
                        t = pC.tile([P, W], bf16, tag=f"vT{h}",
                                    name=f"vT{h}_{G}")
                        act(t, ps, "lin_b1", h)
                        vT.append(t)
                    pso = psO.tile([1, W], f32, tag="pso", name=f"pso{G}")
                    nc.tensor.matmul(
                        pso[:], lw2_sb[0][:], vT[0][:], start=True,
                        stop=False)
                    nc.tensor.matmul(
                        pso[:], lw2_sb[1][:], vT[1][:], start=False,
                        stop=True)
                    nc.scalar.activation(
                        out_row[0:1, G * W:(G + 1) * W], pso[:],
                        Ident, bias=lb2_sb[0:1, 0:1],
                    )

                for G in range(NEG):
                    ai, aj = gath[G]
                    cn = pCn.tile([P, N], fp8, tag="cn", name=f"cn{G}")
                    nc.vector.tensor_tensor(
                        out=cn[:].bitcast(u16),
                        in0=ai[:].bitcast(u16),
                        in1=aj[:].bitcast(u16),
                        op=AND,
                    )
                    cn_v = cn[:].rearrange(
                        "p (ck e j) -> p ck j e", ck=NCHUNK, j=2)
                    for ch in range(2):
                        psb = psB.tile([P, EG], f32, tag="psb",
                                       name=f"psb_{G}{ch}")
                        for ck in range(NCHUNK):
                            nc.tensor.matmul(
                                psb[:], h8_v[:, ck, ch, :, :],
                                cn_v[:, ck, :, :],
                                start=(ck == 0), stop=(ck == NCHUNK - 1),
                                perf_mode=DR,
                            )
                        if G == NEG - 1:
                            nc.vector.tensor_copy(
                                xcnT_sb[ch][:, G * EG:(G + 1) * EG], psb[:])
                        else:
                            nc.scalar.activation(
                                xcnT_sb[ch][:, G * EG:(G + 1) * EG], psb[:],
                                Ident)
                    if G % 2 == 1:
                        stage_c(G // 2)

            nc.sync.dma_start(out=out_d[:, :], in_=out_row[0:1, :])

    # Populate .instr bytes for extended-inst InstISA subclasses (the
    # PseudoReloadLibraryIndex library load + DMAGatherAnt). Raw Bass does
    # not run Bacc's codegen pass; without this walrus sees empty .instr
    # and fails codegen with "ISA wrong length".
    mybir.codegen_inst_isa_subclasses(nc)
    return _split_multi_waits(nc) if split_waits else nc


def _col_shuffle_perm():
    """d[m]: DRAM column position for original node m so the 16-bit
    transposing gather lands bytes exactly in the DoubleRow moving layout
    (node 256*ck + 128*j + p at [p, ck, byte 2e+j])."""
    m = np.arange(N)
    T = m // P
    p = m % P
    return 256 * (T // 2) + 2 * p + (T % 2)


def kernel(**inputs):
    from concourse.bass_utils import run_bass_kernel_spmd

    if "nc" not in _CACHE:
        _CACHE["nc"] = _build()
    nc = _CACHE["nc"]

    x = np.ascontiguousarray(inputs["x"], dtype=np.float32)
    adj8 = np.ascontiguousarray(inputs["adj"]).astype(ml_dtypes.float8_e4m3)
    d = _col_shuffle_perm()
    adjs = np.empty_like(adj8)
    adjs[:, d] = adj8
    tar = np.asarray(inputs["tar_ei"]).astype(np.int16)

    x8 = x.astype(ml_dtypes.float8_e4m3)
    # xa8i[p, 2n+j] = x[n, 128j+p]
    xa8i = np.ascontiguousarray(
        x8.reshape(N, 2, P).transpose(2, 0, 1).reshape(P, 2 * N))
    # xr8t[p, T*C + c] = x[T*128 + p, c]
    xr8t = np.ascontiguousarray(
        x8.reshape(NT, P, C).transpose(1, 0, 2).reshape(P, NT * C))

    def wtile(w, dt):
        # [p, (ksub 2, cout C)] from [C, C]
        return np.ascontiguousarray(
            np.asarray(w).astype(dt).reshape(2, P, C).transpose(1, 0, 2)
            .reshape(P, 2 * C))

    w2_8 = np.asarray(inputs["xlin_w2"]).astype(ml_dtypes.float8_e4m3)
    # w2i[p, 2c+j] = W2[128j+p, c]
    w2i = np.ascontiguousarray(
        w2_8.reshape(2, P, C).transpose(1, 2, 0).reshape(P, 2 * C))
    wa8 = np.concatenate(
        [wtile(inputs["xlin_w1"], ml_dtypes.float8_e4m3), w2i], axis=1)
    b2 = np.asarray(inputs["xlin_b2"], np.float32).reshape(1, C)
    onesb2 = np.concatenate(
        [np.ones((1, P), np.float32), b2, b2],
        axis=1).astype(ml_dtypes.float8_e4m3)
    beta_v = float(np.asarray(inputs["beta"]).reshape(-1)[0])
    winp = {n: np.asarray(inputs[n], np.float32) for n in
            ("xcn_w1", "xcn_w2", "xij_w", "lin_w1")}
    winp["xcn_w2"] = winp["xcn_w2"] * beta_v
    wc = np.concatenate(
        [wtile(winp[n], ml_dtypes.bfloat16)
         for n in ("xcn_w1", "xcn_w2", "xij_w", "lin_w1")], axis=1)

    def btile(b):
        return np.ascontiguousarray(
            np.asarray(b, dtype=np.float32).reshape(2, P).T)

    binp = {n: np.asarray(inputs[n], np.float32) for n in
            ("xlin_b1", "xcn_b1", "xcn_b2", "xij_b", "lin_b1")}
    binp["xcn_b2"] = binp["xcn_b2"] * beta_v
    fpk = np.concatenate(
        [btile(binp[n]) for n in
         ("xlin_b1", "xcn_b1", "xcn_b2", "xij_b", "lin_b1")] +
        [np.full((P, 1), beta_v, dtype=np.float32),
         np.full((P, 1), np.asarray(inputs["lin_b2"]).reshape(-1)[0],
                 dtype=np.float32)],
        axis=1)

    common = {
        "x": x.astype(ml_dtypes.bfloat16),
        "xa8i": xa8i,
        "xr8t": xr8t,
        "adjs": adjs,
        "wa8": wa8,
        "onesb2": onesb2,
        "wc": wc,
        "fpk": fpk,
        "lin_w2": np.ascontiguousarray(inputs["lin_w2"]).astype(
            ml_dtypes.bfloat16),
    }

    in_maps = []
    for c in range(NCORES):
        m = dict(common)
        tc_ = tar[:, c * EL:(c + 1) * EL]  # [2, EL]
        idx16 = np.empty((16, 2 * EL // 16), np.int16)
        for w in range(2):
            idx16[:, w * 64:(w + 1) * 64] = tc_[w].reshape(64, 16).T
        m["idx16"] = np.ascontiguousarray(np.tile(idx16, (8, 1)))
        in_maps.append(m)

    res = run_bass_kernel_spmd(
        nc, in_maps, core_ids=list(range(NCORES)), trace=TRACE
    )
    global LAST_RESULT
    LAST_RESULT = res
    out = np.concatenate(
        [res.results[c]["out"].reshape(EL, 1) for c in range(NCORES)], axis=0
    )
    return out.astype(np.float32)
